# revision 13
# baseline (speedup 1.0000x reference)
"""Criss-cross attention (2-stream) Trainium2 kernel.

Data-parallel over batch B=8 across 8 NeuronCores; one image pair per core.

Per-core algorithm (all matmuls bf16, fp32 PSUM accumulation):
  - q/k projections for both streams in one pass (bias host-corrected)
  - transposed logits E^T per column (diag-masked) / per row,
    joint softmax without max-subtraction (logits are O(30); exp safe in f32)
  - Z-trick: Z[c',p] = sum_g x[c',g] * Phat[p,g] using host-supplied
    spatially-transposed x copies, then one dense (gamma*wv) @ Z projection.
    v-bias folds out exactly because joint softmax weights sum to 1:
      attn = wv@Z + bv;  out = gamma*attn + x = (gamma*wv)@Z + (x + gamma*bv)
    with x~ = x + gamma*bv supplied by host (and bq' = bq - wq@(gamma*bv),
    bk' = bk - wk@(gamma*bv) correcting the q/k projections).
"""

import sys

sys.path.insert(0, "/opt/trn_rl_repo")

import numpy as np
import ml_dtypes

BF = ml_dtypes.bfloat16
B, C, H, W = 8, 256, 96, 96
CQ = 32
S = H * W  # 9216
NSL = S // 512  # 18
NEG = -1.0e30
GRP = 8   # logit slices per eps tile (4 per psum bank)
SLW = 12  # spatial slices per xT slab

_CACHE = {}


def build_nc():
    import concourse.tile as tile
    from concourse import bacc, mybir

    f32 = mybir.dt.float32
    bf16 = mybir.dt.bfloat16

    nc = bacc.Bacc("TRN2", target_bir_lowering=False, debug=False, num_devices=8)

    din = {}

    def dparam(name, shape, dt=bf16):
        din[name] = nc.dram_tensor(name, shape, dt, kind="ExternalInput").ap()

    dparam("xa0", [C, S])          # bf16(x0 + g*bv0), channel-major
    dparam("xa1", [C, S])
    dparam("xtc0", [H, W * C])     # xtc[h, w*256+c] = x0[c,h,w]  (raw x)
    dparam("xtr0", [W, H * C])     # xtr[w, h*256+c] = x0[c,h,w]
    dparam("xtc1", [H, W * C])
    dparam("xtr1", [W, H * C])
    dparam("wqk", [C, 128])        # cols: wq0T|wk0T|wq1T|wk1T
    dparam("wv0", [C, C])          # (gamma*wv0).T
    dparam("wv1", [C, C])
    dparam("qkb", [128, 1], f32)   # bq0'|bk0'|bq1'|bk1'
    dparam("mask", [H, H], f32)    # -1e30 on diagonal else 0
    out = nc.dram_tensor("out", [2, C, S], bf16, kind="ExternalOutput").ap()

    with tile.TileContext(nc) as tc:
        _emit(tc, nc, din, out, mybir)

    nc.compile()
    return nc


def _emit(tc, nc, din, out, mybir):
    from contextlib import ExitStack

    f32 = mybir.dt.float32
    bf16 = mybir.dt.bfloat16
    EXP = mybir.ActivationFunctionType.Exp
    LN = mybir.ActivationFunctionType.Ln
    CPY = mybir.ActivationFunctionType.Copy
    ADD = mybir.AluOpType.add
    MUL = mybir.AluOpType.mult

    ctx = ExitStack()
    with ctx:
        const = ctx.enter_context(tc.tile_pool(name="const", bufs=1))
        persist = ctx.enter_context(tc.tile_pool(name="persist", bufs=1))
        slab = ctx.enter_context(tc.tile_pool(name="slab", bufs=2))
        tsl = ctx.enter_context(tc.tile_pool(name="tsl", bufs=1))
        obuf = ctx.enter_context(tc.tile_pool(name="obuf", bufs=2))
        resl = ctx.enter_context(tc.tile_pool(name="resl", bufs=2))
        # PSUM pools: eps 1x2 banks + lps 1x2 + ps 3x1 = 7 of 8 banks
        eps = ctx.enter_context(tc.tile_pool(name="eps", bufs=1, space="PSUM"))
        lps = ctx.enter_context(tc.tile_pool(name="lps", bufs=1, space="PSUM"))
        ps = ctx.enter_context(tc.tile_pool(name="ps", bufs=3, space="PSUM"))

        # ---------------- constants ----------------
        wqk_t = []
        for kc in range(2):
            t = const.tile([128, 128], bf16, tag=f"wqk{kc}")
            nc.scalar.dma_start(t[:], din["wqk"][kc * 128:(kc + 1) * 128, :])
            wqk_t.append(t)
        wv_t = [[[None] * 2 for _ in range(2)] for _ in range(2)]
        for s in range(2):
            for kc in range(2):
                for mc in range(2):
                    t = const.tile([128, 128], bf16, tag=f"wv{s}{kc}{mc}")
                    nc.scalar.dma_start(
                        t[:],
                        din[f"wv{s}"][kc * 128:(kc + 1) * 128,
                                      mc * 128:(mc + 1) * 128],
                    )
                    wv_t[s][kc][mc] = t
        qkb_t = const.tile([128, 1], f32, tag="qkb")
        nc.scalar.dma_start(qkb_t[:], din["qkb"][:])
        mask_t = const.tile([H, H], f32, tag="mask")
        nc.scalar.dma_start(mask_t[:], din["mask"][:])
        ones_t = const.tile([H, H], bf16, tag="ones")
        nc.vector.memset(ones_t[:], 1.0)

        # ---------------- q/k projections (xa streamed in slabs) --------
        # qk_t rows: q0(0:32) k0(32:64) q1(64:96) k1(96:128)
        qk_t = persist.tile([128, S], bf16, tag="qk")
        for n0 in range(0, NSL, 2):
            xsl = [[None] * 2 for _ in range(2)]
            for s in range(2):
                for kc in range(2):
                    t = resl.tile(
                        [128, 1024], bf16, tag=f"x{s}s{kc}", name=f"x{s}s{kc}"
                    )
                    nc.sync.dma_start(
                        t[:],
                        din[f"xa{s}"][kc * 128:(kc + 1) * 128,
                                      n0 * 512:(n0 + 2) * 512],
                    )
                    xsl[s][kc] = t
            for j in range(2):
                n = n0 + j
                sl = slice(n * 512, (n + 1) * 512)
                jsl = slice(j * 512, (j + 1) * 512)
                p = ps.tile([128, 512], f32, tag="ps")
                for kc in range(2):
                    nc.tensor.matmul(
                        p[0:64, :], wqk_t[kc][:, 0:64], xsl[0][kc][:, jsl],
                        start=(kc == 0), stop=(kc == 1),
                    )
                for kc in range(2):
                    nc.tensor.matmul(
                        p[64:128, :], wqk_t[kc][:, 64:128], xsl[1][kc][:, jsl],
                        start=(kc == 0), stop=(kc == 1),
                        tile_position=(0, 64), skip_group_check=True,
                    )
                nc.vector.tensor_scalar_add(qk_t[:, sl], p[:], qkb_t[:])

        # ---------------- attends ----------------
        for a in range(2):
            kvs = a
            qr = 64 if a == 0 else 0    # query rows (q1 / q0)
            kr = 32 if a == 0 else 96   # key rows (k0 / k1)

            # PE requires matmul operands at equal base partitions: copy the
            # attend's q/k blocks to partition-0-based tiles (SBUF->SBUF DMA).
            qs = persist.tile([32, S], bf16, tag="qs")
            ks = persist.tile([32, S], bf16, tag="ks")
            nc.scalar.dma_start(qs[:], qk_t[qr:qr + 32, :])
            nc.scalar.dma_start(ks[:], qk_t[kr:kr + 32, :])
            qs_wh = qs[:].rearrange("p (h w) -> p w h", w=W)
            ks_wh = ks[:].rearrange("p (h w) -> p w h", w=W)

            pcol = persist.tile([H, S], bf16, tag="pcol")
            prow = persist.tile([W, S], bf16, tag="prow")
            # pixel (h,w) lives at free index w*96+h in pcol/prow/rrep

            # ---- column-branch logits + mask + exp ----
            for w0 in range(0, W, GRP):
                et = eps.tile([128, 1024], f32, tag="eps")
                for j in range(GRP):
                    w = w0 + j
                    off = (j // 4) * 512 + (j % 4) * 96
                    nc.tensor.matmul(
                        et[0:96, off:off + 96],
                        ks_wh[:, w, :],
                        qs_wh[:, w, :],
                        start=True, stop=True, skip_group_check=True,
                    )
                for bk in range(2):
                    io = et[0:96, bk * 512:bk * 512 + 384]
                    nc.vector.tensor_tensor(
                        io, io,
                        mask_t[:].unsqueeze(1).broadcast_to((96, 4, 96)),
                        ADD,
                    )
                src = et[0:96, :].rearrange("p (b x) -> p b x", x=512)[:, :, 0:384]
                nc.scalar.activation(
                    pcol[:, w0 * 96:(w0 + GRP) * 96], src, EXP
                )

            # ---- row-branch logits + exp (strided scatter) ----
            prow_hw = prow[:].rearrange("p (w h) -> p h w", h=H)
            for h0 in range(0, H, GRP):
                et = eps.tile([128, 1024], f32, tag="eps")
                for j in range(GRP):
                    h = h0 + j
                    off = (j // 4) * 512 + (j % 4) * 96
                    nc.tensor.matmul(
                        et[0:96, off:off + 96],
                        ks[:, h * 96:(h + 1) * 96],
                        qs[:, h * 96:(h + 1) * 96],
                        start=True, stop=True, skip_group_check=True,
                    )
                for bk in range(2):
                    nc.scalar.activation(
                        prow_hw[:, h0 + bk * 4:h0 + bk * 4 + 4, :],
                        et[0:96, bk * 512:bk * 512 + 384],
                        EXP,
                    )

            # ---- stats: l = colsum+rowsum replicated; r = exp(-ln l) ----
            rrep = persist.tile([H, S], bf16, tag="rrep")
            for n0 in range(0, NSL, 2):
                lt = lps.tile([96, 1024], f32, tag="lps")
                for j in range(2):
                    sl = slice((n0 + j) * 512, (n0 + j + 1) * 512)
                    nc.tensor.matmul(
                        lt[:, j * 512:(j + 1) * 512], ones_t[:], pcol[:, sl],
                        start=True, stop=False, skip_group_check=True,
                    )
                    nc.tensor.matmul(
                        lt[:, j * 512:(j + 1) * 512], ones_t[:], prow[:, sl],
                        start=False, stop=True, skip_group_check=True,
                    )
                tl = tsl.tile([96, 1024], f32, tag="tln")
                nc.scalar.activation(tl[:], lt[:], LN)
                nc.scalar.activation(
                    rrep[:, n0 * 512:(n0 + 2) * 512], tl[:], EXP, scale=-1.0
                )

            # ---- normalize P in place (gpsimd; keeps DVE free) ----
            nc.gpsimd.tensor_tensor(pcol[:], pcol[:], rrep[:], MUL)
            nc.gpsimd.tensor_tensor(prow[:], prow[:], rrep[:], MUL)

            # ---- Z accumulation ----
            z = [
                persist.tile([128, S], bf16, tag=f"z{kc}", name=f"z{kc}")
                for kc in range(2)
            ]
            z_wh = [zz[:].rearrange("p (h w) -> p w h", w=W) for zz in z]
            xtc = din[f"xtc{kvs}"][:].rearrange("p (w c) -> p w c", c=C)
            xtr = din[f"xtr{kvs}"][:].rearrange("p (h c) -> p h c", c=C)

            # column branch: per w, Z[c', h]; scatter w-strided into z
            for w0 in range(0, W, SLW):
                xs = slab.tile([H, SLW * 256], bf16, tag="xslab")
                nc.gpsimd.dma_start(xs[:], xtc[:, w0:w0 + SLW, :])
                for kc in range(2):
                    for j0 in range(0, SLW, 4):
                        zp = ps.tile([128, 512], f32, tag="ps")
                        for j in range(4):
                            wl = j0 + j
                            nc.tensor.matmul(
                                zp[:, j * 96:(j + 1) * 96],
                                xs[:, wl * 256 + kc * 128:wl * 256 + kc * 128 + 128],
                                pcol[:, (w0 + wl) * 96:(w0 + wl + 1) * 96],
                                start=True, stop=True, skip_group_check=True,
                            )
                        nc.scalar.activation(
                            z_wh[kc][:, w0 + j0:w0 + j0 + 4, :],
                            zp[:, 0:384], CPY,
                        )
            # row branch: per h, Z[c', w]; accumulate into z
            for h0 in range(0, H, SLW):
                xs = slab.tile([W, SLW * 256], bf16, tag="xslab")
                nc.gpsimd.dma_start(xs[:], xtr[:, h0:h0 + SLW, :])
                for kc in range(2):
                    for j0 in range(0, SLW, 4):
                        zp = ps.tile([128, 512], f32, tag="ps")
                        for j in range(4):
                            hl = j0 + j
                            nc.tensor.matmul(
                                zp[:, j * 96:(j + 1) * 96],
                                xs[:, hl * 256 + kc * 128:hl * 256 + kc * 128 + 128],
                                prow_hw[:, h0 + hl, :],
                                start=True, stop=True, skip_group_check=True,
                            )
                        zsl = z[kc][:, (h0 + j0) * 96:(h0 + j0 + 4) * 96]
                        nc.vector.tensor_tensor(zsl, zp[:, 0:384], zsl, ADD)

            # ---- final projection + residual + store ----
            for mc in range(2):
                for n0 in range(0, NSL, 6):
                    ob = obuf.tile([128, 3072], bf16, tag="ob")
                    rt = resl.tile([128, 3072], bf16, tag="res")
                    nc.sync.dma_start(
                        rt[:],
                        din[f"xa{kvs}"][mc * 128:(mc + 1) * 128,
                                        n0 * 512:(n0 + 6) * 512],
                    )
                    for j in range(6):
                        n = n0 + j
                        sl = slice(n * 512, (n + 1) * 512)
                        jsl = slice(j * 512, (j + 1) * 512)
                        op = ps.tile([128, 512], f32, tag="ps")
                        for kc in range(2):
                            nc.tensor.matmul(
                                op[:], wv_t[kvs][kc][mc], z[kc][:, sl],
                                start=(kc == 0), stop=(kc == 1),
                            )
                        nc.vector.tensor_tensor(
                            ob[:, jsl], op[:], rt[:, jsl], ADD
                        )
                    nc.scalar.dma_start(
                        out[a, mc * 128:(mc + 1) * 128,
                            n0 * 512:(n0 + 6) * 512],
                        ob[:],
                    )


def prep_inputs(inputs):
    """Host-side per-core input prep (numpy)."""
    g = float(np.asarray(inputs["gamma"]).reshape(-1)[0])
    mask = np.zeros((H, H), np.float32)
    np.fill_diagonal(mask, NEG)
    wqk = np.concatenate(
        [inputs["wq0"].T, inputs["wk0"].T, inputs["wq1"].T, inputs["wk1"].T],
        axis=1,
    ).astype(BF)
    wv0 = (g * np.asarray(inputs["wv0"], np.float64)).T.astype(BF)
    wv1 = (g * np.asarray(inputs["wv1"], np.float64)).T.astype(BF)
    gb0 = g * np.asarray(inputs["bv0"], np.float64)
    gb1 = g * np.asarray(inputs["bv1"], np.float64)
    qkb = np.concatenate(
        [
            inputs["bq0"] - inputs["wq0"].astype(np.float64) @ gb0,
            inputs["bk0"] - inputs["wk0"].astype(np.float64) @ gb0,
            inputs["bq1"] - inputs["wq1"].astype(np.float64) @ gb1,
            inputs["bk1"] - inputs["wk1"].astype(np.float64) @ gb1,
        ]
    ).astype(np.float32)[:, None]
    maps = []
    for b in range(B):
        x0 = np.asarray(inputs["x0"][b], np.float32)
        x1 = np.asarray(inputs["x1"][b], np.float32)
        maps.append({
            "xa0": (x0 + np.float32(gb0[:, None, None])).reshape(C, S).astype(BF),
            "xa1": (x1 + np.float32(gb1[:, None, None])).reshape(C, S).astype(BF),
            "xtc0": np.ascontiguousarray(x0.transpose(1, 2, 0)).reshape(H, W * C).astype(BF),
            "xtr0": np.ascontiguousarray(x0.transpose(2, 1, 0)).reshape(W, H * C).astype(BF),
            "xtc1": np.ascontiguousarray(x1.transpose(1, 2, 0)).reshape(H, W * C).astype(BF),
            "xtr1": np.ascontiguousarray(x1.transpose(2, 1, 0)).reshape(W, H * C).astype(BF),
            "wqk": wqk, "wv0": wv0, "wv1": wv1, "qkb": qkb, "mask": mask,
        })
    return maps


def postprocess(results):
    cat0 = np.empty((B, C, H, W), np.float32)
    cat1 = np.empty((B, C, H, W), np.float32)
    for b in range(B):
        o = np.asarray(results[b]["out"]).astype(np.float32).reshape(2, C, H, W)
        cat0[b] = o[0]
        cat1[b] = o[1]
    return (cat0, cat1)


def kernel(**inputs):
    from concourse.bass_utils import run_bass_kernel_spmd

    if "nc" not in _CACHE:
        _CACHE["nc"] = build_nc()
    nc = _CACHE["nc"]
    maps = prep_inputs(inputs)
    res = run_bass_kernel_spmd(nc, maps, core_ids=list(range(B)))
    return postprocess(res.results)


# revision 15
# speedup vs baseline: 136.9027x; 136.9027x over previous
"""Criss-cross attention (2-stream) Trainium2 kernel.

Data-parallel over batch B=8 across 8 NeuronCores; one image pair per core.

Per-core algorithm (all matmuls bf16, fp32 PSUM accumulation):
  - q/k projections for both streams in one pass (bias host-corrected)
  - transposed logits E^T per column (diag-masked) / per row,
    joint softmax without max-subtraction (logits are O(30); exp safe in f32)
  - Z-trick: Z[c',p] = sum_g x[c',g] * Phat[p,g] using host-supplied
    spatially-transposed x copies, then one dense (gamma*wv) @ Z projection.
    v-bias folds out exactly because joint softmax weights sum to 1:
      attn = wv@Z + bv;  out = gamma*attn + x = (gamma*wv)@Z + (x + gamma*bv)
    with x~ = x + gamma*bv supplied by host (and bq' = bq - wq@(gamma*bv),
    bk' = bk - wk@(gamma*bv) correcting the q/k projections).
"""

import sys

sys.path.insert(0, "/opt/trn_rl_repo")

import numpy as np
import ml_dtypes

BF = ml_dtypes.bfloat16
B, C, H, W = 8, 256, 96, 96
CQ = 32
S = H * W  # 9216
NSL = S // 512  # 18
NEG = -1.0e30
GRP = 8   # logit slices per eps tile (4 per psum bank)
SLW = 12  # spatial slices per xT slab

_CACHE = {}


def build_nc(reps=1):
    import concourse.tile as tile
    from concourse import bacc, mybir

    f32 = mybir.dt.float32
    bf16 = mybir.dt.bfloat16

    nc = bacc.Bacc("TRN2", target_bir_lowering=False, debug=False, num_devices=8)

    din = {}

    def dparam(name, shape, dt=bf16):
        din[name] = nc.dram_tensor(name, shape, dt, kind="ExternalInput").ap()

    dparam("xa0", [C, S])          # bf16(x0 + g*bv0), channel-major
    dparam("xa1", [C, S])
    dparam("xtc0", [H, W * C])     # xtc[h, w*256+c] = x0[c,h,w]  (raw x)
    dparam("xtr0", [W, H * C])     # xtr[w, h*256+c] = x0[c,h,w]
    dparam("xtc1", [H, W * C])
    dparam("xtr1", [W, H * C])
    dparam("wqk", [C, 128])        # cols: wq0T|wk0T|wq1T|wk1T
    dparam("wv0", [C, C])          # (gamma*wv0).T
    dparam("wv1", [C, C])
    dparam("qkb", [128, 1], f32)   # bq0'|bk0'|bq1'|bk1'
    dparam("mask", [H, H], f32)    # -1e30 on diagonal else 0
    out = nc.dram_tensor("out", [2, C, S], bf16, kind="ExternalOutput").ap()

    with tile.TileContext(nc) as tc:
        if reps == 1:
            _emit(tc, nc, din, out, mybir)
        else:
            with tc.For_i(0, reps, 1):
                _emit(tc, nc, din, out, mybir)

    nc.compile()
    return nc


def _emit(tc, nc, din, out, mybir):
    from contextlib import ExitStack

    f32 = mybir.dt.float32
    bf16 = mybir.dt.bfloat16
    EXP = mybir.ActivationFunctionType.Exp
    LN = mybir.ActivationFunctionType.Ln
    CPY = mybir.ActivationFunctionType.Copy
    ADD = mybir.AluOpType.add
    MUL = mybir.AluOpType.mult

    ctx = ExitStack()
    with ctx:
        const = ctx.enter_context(tc.tile_pool(name="const", bufs=1))
        persist = ctx.enter_context(tc.tile_pool(name="persist", bufs=1))
        slab = ctx.enter_context(tc.tile_pool(name="slab", bufs=2))
        tsl = ctx.enter_context(tc.tile_pool(name="tsl", bufs=1))
        obuf = ctx.enter_context(tc.tile_pool(name="obuf", bufs=2))
        resl = ctx.enter_context(tc.tile_pool(name="resl", bufs=2))
        # PSUM pools: eps 1x2 banks + lps 1x2 + ps 3x1 = 7 of 8 banks
        eps = ctx.enter_context(tc.tile_pool(name="eps", bufs=1, space="PSUM"))
        lps = ctx.enter_context(tc.tile_pool(name="lps", bufs=1, space="PSUM"))
        ps = ctx.enter_context(tc.tile_pool(name="ps", bufs=3, space="PSUM"))

        # ---------------- constants ----------------
        wqk_t = []
        for kc in range(2):
            t = const.tile([128, 128], bf16, tag=f"wqk{kc}")
            nc.scalar.dma_start(t[:], din["wqk"][kc * 128:(kc + 1) * 128, :])
            wqk_t.append(t)
        wv_t = [[[None] * 2 for _ in range(2)] for _ in range(2)]
        for s in range(2):
            for kc in range(2):
                for mc in range(2):
                    t = const.tile([128, 128], bf16, tag=f"wv{s}{kc}{mc}")
                    nc.scalar.dma_start(
                        t[:],
                        din[f"wv{s}"][kc * 128:(kc + 1) * 128,
                                      mc * 128:(mc + 1) * 128],
                    )
                    wv_t[s][kc][mc] = t
        qkb_t = const.tile([128, 1], f32, tag="qkb")
        nc.scalar.dma_start(qkb_t[:], din["qkb"][:])
        mask_t = const.tile([H, H], f32, tag="mask")
        nc.scalar.dma_start(mask_t[:], din["mask"][:])
        ones_t = const.tile([H, H], bf16, tag="ones")
        nc.vector.memset(ones_t[:], 1.0)

        # ---------------- q/k projections (xa streamed in slabs) --------
        # qk_t rows: q0(0:32) k0(32:64) q1(64:96) k1(96:128)
        qk_t = persist.tile([128, S], bf16, tag="qk")
        for n0 in range(0, NSL, 2):
            xsl = [[None] * 2 for _ in range(2)]
            for s in range(2):
                for kc in range(2):
                    t = resl.tile(
                        [128, 1024], bf16, tag=f"x{s}s{kc}", name=f"x{s}s{kc}"
                    )
                    nc.sync.dma_start(
                        t[:],
                        din[f"xa{s}"][kc * 128:(kc + 1) * 128,
                                      n0 * 512:(n0 + 2) * 512],
                    )
                    xsl[s][kc] = t
            for j in range(2):
                n = n0 + j
                sl = slice(n * 512, (n + 1) * 512)
                jsl = slice(j * 512, (j + 1) * 512)
                p = ps.tile([128, 512], f32, tag="ps")
                for kc in range(2):
                    nc.tensor.matmul(
                        p[0:64, :], wqk_t[kc][:, 0:64], xsl[0][kc][:, jsl],
                        start=(kc == 0), stop=(kc == 1),
                    )
                for kc in range(2):
                    nc.tensor.matmul(
                        p[64:128, :], wqk_t[kc][:, 64:128], xsl[1][kc][:, jsl],
                        start=(kc == 0), stop=(kc == 1),
                        tile_position=(0, 64), skip_group_check=True,
                    )
                nc.vector.tensor_scalar_add(qk_t[:, sl], p[:], qkb_t[:])

        # ---------------- attends ----------------
        for a in range(2):
            kvs = a
            qr = 64 if a == 0 else 0    # query rows (q1 / q0)
            kr = 32 if a == 0 else 96   # key rows (k0 / k1)

            # PE requires matmul operands at equal base partitions: copy the
            # attend's q/k blocks to partition-0-based tiles (SBUF->SBUF DMA).
            qs = persist.tile([32, S], bf16, tag="qs")
            ks = persist.tile([32, S], bf16, tag="ks")
            nc.scalar.dma_start(qs[:], qk_t[qr:qr + 32, :])
            nc.scalar.dma_start(ks[:], qk_t[kr:kr + 32, :])
            qs_wh = qs[:].rearrange("p (h w) -> p w h", w=W)
            ks_wh = ks[:].rearrange("p (h w) -> p w h", w=W)

            pcol = persist.tile([H, S], bf16, tag="pcol")
            prow = persist.tile([W, S], bf16, tag="prow")
            # pixel (h,w) lives at free index w*96+h in pcol/prow/rrep

            # ---- column-branch logits + mask + exp ----
            for w0 in range(0, W, GRP):
                et = eps.tile([128, 1024], f32, tag="eps")
                for j in range(GRP):
                    w = w0 + j
                    off = (j // 4) * 512 + (j % 4) * 96
                    nc.tensor.matmul(
                        et[0:96, off:off + 96],
                        ks_wh[:, w, :],
                        qs_wh[:, w, :],
                        start=True, stop=True, skip_group_check=True,
                    )
                for bk in range(2):
                    io = et[0:96, bk * 512:bk * 512 + 384]
                    nc.vector.tensor_tensor(
                        io, io,
                        mask_t[:].unsqueeze(1).broadcast_to((96, 4, 96)),
                        ADD,
                    )
                src = et[0:96, :].rearrange("p (b x) -> p b x", x=512)[:, :, 0:384]
                nc.scalar.activation(
                    pcol[:, w0 * 96:(w0 + GRP) * 96], src, EXP
                )

            # ---- row-branch logits + exp (strided scatter) ----
            prow_hw = prow[:].rearrange("p (w h) -> p h w", h=H)
            for h0 in range(0, H, GRP):
                et = eps.tile([128, 1024], f32, tag="eps")
                for j in range(GRP):
                    h = h0 + j
                    off = (j // 4) * 512 + (j % 4) * 96
                    nc.tensor.matmul(
                        et[0:96, off:off + 96],
                        ks[:, h * 96:(h + 1) * 96],
                        qs[:, h * 96:(h + 1) * 96],
                        start=True, stop=True, skip_group_check=True,
                    )
                for bk in range(2):
                    nc.scalar.activation(
                        prow_hw[:, h0 + bk * 4:h0 + bk * 4 + 4, :],
                        et[0:96, bk * 512:bk * 512 + 384],
                        EXP,
                    )

            # ---- stats: l = colsum+rowsum replicated; r = exp(-ln l) ----
            rrep = persist.tile([H, S], bf16, tag="rrep")
            for n0 in range(0, NSL, 2):
                lt = lps.tile([96, 1024], f32, tag="lps")
                for j in range(2):
                    sl = slice((n0 + j) * 512, (n0 + j + 1) * 512)
                    nc.tensor.matmul(
                        lt[:, j * 512:(j + 1) * 512], ones_t[:], pcol[:, sl],
                        start=True, stop=False, skip_group_check=True,
                    )
                    nc.tensor.matmul(
                        lt[:, j * 512:(j + 1) * 512], ones_t[:], prow[:, sl],
                        start=False, stop=True, skip_group_check=True,
                    )
                tl = tsl.tile([96, 1024], f32, tag="tln")
                nc.scalar.activation(tl[:], lt[:], LN)
                nc.scalar.activation(
                    rrep[:, n0 * 512:(n0 + 2) * 512], tl[:], EXP, scale=-1.0
                )

            # ---- normalize P in place (gpsimd; keeps DVE free) ----
            nc.gpsimd.tensor_tensor(pcol[:], pcol[:], rrep[:], MUL)
            nc.gpsimd.tensor_tensor(prow[:], prow[:], rrep[:], MUL)

            # ---- Z accumulation ----
            z = [
                persist.tile([128, S], bf16, tag=f"z{kc}", name=f"z{kc}")
                for kc in range(2)
            ]
            z_wh = [zz[:].rearrange("p (h w) -> p w h", w=W) for zz in z]
            xtc = din[f"xtc{kvs}"][:].rearrange("p (w c) -> p w c", c=C)
            xtr = din[f"xtr{kvs}"][:].rearrange("p (h c) -> p h c", c=C)

            # column branch: per w, Z[c', h]; scatter w-strided into z
            for w0 in range(0, W, SLW):
                xs = slab.tile([H, SLW * 256], bf16, tag="xslab")
                nc.gpsimd.dma_start(xs[:], xtc[:, w0:w0 + SLW, :])
                for kc in range(2):
                    for j0 in range(0, SLW, 4):
                        zp = ps.tile([128, 512], f32, tag="ps")
                        for j in range(4):
                            wl = j0 + j
                            nc.tensor.matmul(
                                zp[:, j * 96:(j + 1) * 96],
                                xs[:, wl * 256 + kc * 128:wl * 256 + kc * 128 + 128],
                                pcol[:, (w0 + wl) * 96:(w0 + wl + 1) * 96],
                                start=True, stop=True, skip_group_check=True,
                            )
                        nc.scalar.activation(
                            z_wh[kc][:, w0 + j0:w0 + j0 + 4, :],
                            zp[:, 0:384], CPY,
                        )
            # row branch: per h, Z[c', w]; accumulate into z
            for h0 in range(0, H, SLW):
                xs = slab.tile([W, SLW * 256], bf16, tag="xslab")
                nc.gpsimd.dma_start(xs[:], xtr[:, h0:h0 + SLW, :])
                for kc in range(2):
                    for j0 in range(0, SLW, 4):
                        zp = ps.tile([128, 512], f32, tag="ps")
                        for j in range(4):
                            hl = j0 + j
                            nc.tensor.matmul(
                                zp[:, j * 96:(j + 1) * 96],
                                xs[:, hl * 256 + kc * 128:hl * 256 + kc * 128 + 128],
                                prow_hw[:, h0 + hl, :],
                                start=True, stop=True, skip_group_check=True,
                            )
                        zsl = z[kc][:, (h0 + j0) * 96:(h0 + j0 + 4) * 96]
                        nc.vector.tensor_tensor(zsl, zp[:, 0:384], zsl, ADD)

            # ---- final projection + residual + store ----
            for mc in range(2):
                for n0 in range(0, NSL, 6):
                    ob = obuf.tile([128, 3072], bf16, tag="ob")
                    rt = resl.tile([128, 3072], bf16, tag="res")
                    nc.sync.dma_start(
                        rt[:],
                        din[f"xa{kvs}"][mc * 128:(mc + 1) * 128,
                                        n0 * 512:(n0 + 6) * 512],
                    )
                    for j in range(6):
                        n = n0 + j
                        sl = slice(n * 512, (n + 1) * 512)
                        jsl = slice(j * 512, (j + 1) * 512)
                        op = ps.tile([128, 512], f32, tag="ps")
                        for kc in range(2):
                            nc.tensor.matmul(
                                op[:], wv_t[kvs][kc][mc], z[kc][:, sl],
                                start=(kc == 0), stop=(kc == 1),
                            )
                        nc.vector.tensor_tensor(
                            ob[:, jsl], op[:], rt[:, jsl], ADD
                        )
                    nc.scalar.dma_start(
                        out[a, mc * 128:(mc + 1) * 128,
                            n0 * 512:(n0 + 6) * 512],
                        ob[:],
                    )


def prep_inputs(inputs):
    """Host-side per-core input prep (numpy)."""
    g = float(np.asarray(inputs["gamma"]).reshape(-1)[0])
    mask = np.zeros((H, H), np.float32)
    np.fill_diagonal(mask, NEG)
    wqk = np.concatenate(
        [inputs["wq0"].T, inputs["wk0"].T, inputs["wq1"].T, inputs["wk1"].T],
        axis=1,
    ).astype(BF)
    wv0 = (g * np.asarray(inputs["wv0"], np.float64)).T.astype(BF)
    wv1 = (g * np.asarray(inputs["wv1"], np.float64)).T.astype(BF)
    gb0 = g * np.asarray(inputs["bv0"], np.float64)
    gb1 = g * np.asarray(inputs["bv1"], np.float64)
    qkb = np.concatenate(
        [
            inputs["bq0"] - inputs["wq0"].astype(np.float64) @ gb0,
            inputs["bk0"] - inputs["wk0"].astype(np.float64) @ gb0,
            inputs["bq1"] - inputs["wq1"].astype(np.float64) @ gb1,
            inputs["bk1"] - inputs["wk1"].astype(np.float64) @ gb1,
        ]
    ).astype(np.float32)[:, None]
    maps = []
    for b in range(B):
        x0 = np.asarray(inputs["x0"][b], np.float32)
        x1 = np.asarray(inputs["x1"][b], np.float32)
        maps.append({
            "xa0": (x0 + np.float32(gb0[:, None, None])).reshape(C, S).astype(BF),
            "xa1": (x1 + np.float32(gb1[:, None, None])).reshape(C, S).astype(BF),
            "xtc0": np.ascontiguousarray(x0.transpose(1, 2, 0)).reshape(H, W * C).astype(BF),
            "xtr0": np.ascontiguousarray(x0.transpose(2, 1, 0)).reshape(W, H * C).astype(BF),
            "xtc1": np.ascontiguousarray(x1.transpose(1, 2, 0)).reshape(H, W * C).astype(BF),
            "xtr1": np.ascontiguousarray(x1.transpose(2, 1, 0)).reshape(W, H * C).astype(BF),
            "wqk": wqk, "wv0": wv0, "wv1": wv1, "qkb": qkb, "mask": mask,
        })
    return maps


def postprocess(results):
    cat0 = np.empty((B, C, H, W), np.float32)
    cat1 = np.empty((B, C, H, W), np.float32)
    for b in range(B):
        o = np.asarray(results[b]["out"]).astype(np.float32).reshape(2, C, H, W)
        cat0[b] = o[0]
        cat1[b] = o[1]
    return (cat0, cat1)


def kernel(**inputs):
    from concourse.bass_utils import run_bass_kernel_spmd

    if "nc" not in _CACHE:
        _CACHE["nc"] = build_nc()
    nc = _CACHE["nc"]
    maps = prep_inputs(inputs)
    res = run_bass_kernel_spmd(nc, maps, core_ids=list(range(B)))
    return postprocess(res.results)


# revision 17
# speedup vs baseline: 146.8449x; 1.0726x over previous
"""Criss-cross attention (2-stream) Trainium2 kernel.

Data-parallel over batch B=8 across 8 NeuronCores; one image pair per core.

Per-core algorithm (all matmuls bf16, fp32 PSUM accumulation):
  - q/k projections for both streams in one pass (bias host-corrected)
  - transposed logits E^T per column (diag-masked) / per row,
    joint softmax without max-subtraction (logits are O(30); exp safe in f32)
  - Z-trick: Z[c',p] = sum_g x[c',g] * Phat[p,g] using host-supplied
    spatially-transposed x copies, then one dense (gamma*wv) @ Z projection.
    v-bias folds out exactly because joint softmax weights sum to 1:
      attn = wv@Z + bv;  out = gamma*attn + x = (gamma*wv)@Z + (x + gamma*bv)
    with x~ = x + gamma*bv supplied by host (and bq' = bq - wq@(gamma*bv),
    bk' = bk - wk@(gamma*bv) correcting the q/k projections).
"""

import sys

sys.path.insert(0, "/opt/trn_rl_repo")

import numpy as np
import ml_dtypes

BF = ml_dtypes.bfloat16
B, C, H, W = 8, 256, 96, 96
CQ = 32
S = H * W  # 9216
NSL = S // 512  # 18
NEG = -1.0e30
GRP = 8   # logit slices per eps tile (4 per psum bank)
SLW = 12  # spatial slices per xT slab

_CACHE = {}


def build_nc(reps=1):
    import concourse.tile as tile
    from concourse import bacc, mybir

    f32 = mybir.dt.float32
    bf16 = mybir.dt.bfloat16

    nc = bacc.Bacc("TRN2", target_bir_lowering=False, debug=False, num_devices=8)

    din = {}

    def dparam(name, shape, dt=bf16):
        din[name] = nc.dram_tensor(name, shape, dt, kind="ExternalInput").ap()

    dparam("xa0", [C, S])          # bf16(x0 + g*bv0), channel-major
    dparam("xa1", [C, S])
    dparam("xtc0", [H, W * C])     # xtc[h, w*256+c] = x0[c,h,w]  (raw x)
    dparam("xtr0", [W, H * C])     # xtr[w, h*256+c] = x0[c,h,w]
    dparam("xtc1", [H, W * C])
    dparam("xtr1", [W, H * C])
    dparam("wqk", [C, 128])        # cols: wq0T|wk0T|wq1T|wk1T
    dparam("wv0", [C, C])          # (gamma*wv0).T
    dparam("wv1", [C, C])
    dparam("qkb", [128, 1], f32)   # bq0'|bk0'|bq1'|bk1'
    dparam("mask", [H, H], f32)    # -1e30 on diagonal else 0
    out = nc.dram_tensor("out", [2, C, S], bf16, kind="ExternalOutput").ap()

    with tile.TileContext(nc) as tc:
        if reps == 1:
            _emit(tc, nc, din, out, mybir)
        else:
            with tc.For_i(0, reps, 1):
                _emit(tc, nc, din, out, mybir)

    nc.compile()
    return nc


def _emit(tc, nc, din, out, mybir):
    from contextlib import ExitStack

    f32 = mybir.dt.float32
    bf16 = mybir.dt.bfloat16
    EXP = mybir.ActivationFunctionType.Exp
    LN = mybir.ActivationFunctionType.Ln
    CPY = mybir.ActivationFunctionType.Copy
    ADD = mybir.AluOpType.add
    MUL = mybir.AluOpType.mult

    ctx = ExitStack()
    with ctx:
        const = ctx.enter_context(tc.tile_pool(name="const", bufs=1))
        persist = ctx.enter_context(tc.tile_pool(name="persist", bufs=1))
        slab = ctx.enter_context(tc.tile_pool(name="slab", bufs=3))
        tsl = ctx.enter_context(tc.tile_pool(name="tsl", bufs=2))
        obuf = ctx.enter_context(tc.tile_pool(name="obuf", bufs=2))
        resl = ctx.enter_context(tc.tile_pool(name="resl", bufs=2))
        # PSUM pools: eps 1x2 banks + lps 1x2 + ps 3x1 = 7 of 8 banks
        eps = ctx.enter_context(tc.tile_pool(name="eps", bufs=2, space="PSUM"))
        lps = ctx.enter_context(tc.tile_pool(name="lps", bufs=1, space="PSUM"))
        ps = ctx.enter_context(tc.tile_pool(name="ps", bufs=2, space="PSUM"))

        # ---------------- constants ----------------
        wqk_t = []
        for kc in range(2):
            t = const.tile([128, 128], bf16, tag=f"wqk{kc}")
            nc.scalar.dma_start(t[:], din["wqk"][kc * 128:(kc + 1) * 128, :])
            wqk_t.append(t)
        wv_t = [[[None] * 2 for _ in range(2)] for _ in range(2)]
        for s in range(2):
            for kc in range(2):
                for mc in range(2):
                    t = const.tile([128, 128], bf16, tag=f"wv{s}{kc}{mc}")
                    nc.scalar.dma_start(
                        t[:],
                        din[f"wv{s}"][kc * 128:(kc + 1) * 128,
                                      mc * 128:(mc + 1) * 128],
                    )
                    wv_t[s][kc][mc] = t
        qkb_t = const.tile([128, 1], f32, tag="qkb")
        nc.scalar.dma_start(qkb_t[:], din["qkb"][:])
        mask_t = const.tile([H, H], f32, tag="mask")
        nc.scalar.dma_start(mask_t[:], din["mask"][:])
        ones_t = const.tile([H, H], bf16, tag="ones")
        nc.vector.memset(ones_t[:], 1.0)

        # ---------------- q/k projections (xa streamed in slabs) --------
        # qk_t rows: q0(0:32) k0(32:64) q1(64:96) k1(96:128)
        qk_t = persist.tile([128, S], bf16, tag="qk")
        for n0 in range(0, NSL, 2):
            xsl = [[None] * 2 for _ in range(2)]
            for s in range(2):
                for kc in range(2):
                    t = resl.tile(
                        [128, 1024], bf16, tag=f"x{s}s{kc}", name=f"x{s}s{kc}"
                    )
                    nc.sync.dma_start(
                        t[:],
                        din[f"xa{s}"][kc * 128:(kc + 1) * 128,
                                      n0 * 512:(n0 + 2) * 512],
                    )
                    xsl[s][kc] = t
            for j in range(2):
                n = n0 + j
                sl = slice(n * 512, (n + 1) * 512)
                jsl = slice(j * 512, (j + 1) * 512)
                p = ps.tile([128, 512], f32, tag="ps")
                for kc in range(2):
                    nc.tensor.matmul(
                        p[0:64, :], wqk_t[kc][:, 0:64], xsl[0][kc][:, jsl],
                        start=(kc == 0), stop=(kc == 1),
                    )
                for kc in range(2):
                    nc.tensor.matmul(
                        p[64:128, :], wqk_t[kc][:, 64:128], xsl[1][kc][:, jsl],
                        start=(kc == 0), stop=(kc == 1),
                        tile_position=(0, 64), skip_group_check=True,
                    )
                nc.vector.tensor_scalar_add(qk_t[:, sl], p[:], qkb_t[:])

        # ---------------- attends ----------------
        for a in range(2):
            kvs = a
            qr = 64 if a == 0 else 0    # query rows (q1 / q0)
            kr = 32 if a == 0 else 96   # key rows (k0 / k1)

            # PE requires matmul operands at equal base partitions: copy the
            # attend's q/k blocks to partition-0-based tiles (SBUF->SBUF DMA).
            qs = persist.tile([32, S], bf16, tag="qs")
            ks = persist.tile([32, S], bf16, tag="ks")
            nc.scalar.dma_start(qs[:], qk_t[qr:qr + 32, :])
            nc.scalar.dma_start(ks[:], qk_t[kr:kr + 32, :])
            qs_wh = qs[:].rearrange("p (h w) -> p w h", w=W)
            ks_wh = ks[:].rearrange("p (h w) -> p w h", w=W)

            pcol = persist.tile([H, S], bf16, tag="pcol")
            prow = persist.tile([W, S], bf16, tag="prow")
            # pixel (h,w) lives at free index w*96+h in pcol/prow/rrep

            # ---- column-branch logits + mask + exp ----
            for w0 in range(0, W, GRP):
                et = eps.tile([128, 1024], f32, tag="eps")
                for j in range(GRP):
                    w = w0 + j
                    off = (j // 4) * 512 + (j % 4) * 96
                    nc.tensor.matmul(
                        et[0:96, off:off + 96],
                        ks_wh[:, w, :],
                        qs_wh[:, w, :],
                        start=True, stop=True, skip_group_check=True,
                    )
                for bk in range(2):
                    io = et[0:96, bk * 512:bk * 512 + 384]
                    nc.vector.tensor_tensor(
                        io, io,
                        mask_t[:].unsqueeze(1).broadcast_to((96, 4, 96)),
                        ADD,
                    )
                src = et[0:96, :].rearrange("p (b x) -> p b x", x=512)[:, :, 0:384]
                nc.scalar.activation(
                    pcol[:, w0 * 96:(w0 + GRP) * 96], src, EXP
                )

            # ---- row-branch logits + exp (strided scatter) ----
            prow_hw = prow[:].rearrange("p (w h) -> p h w", h=H)
            for h0 in range(0, H, GRP):
                et = eps.tile([128, 1024], f32, tag="eps")
                for j in range(GRP):
                    h = h0 + j
                    off = (j // 4) * 512 + (j % 4) * 96
                    nc.tensor.matmul(
                        et[0:96, off:off + 96],
                        ks[:, h * 96:(h + 1) * 96],
                        qs[:, h * 96:(h + 1) * 96],
                        start=True, stop=True, skip_group_check=True,
                    )
                for bk in range(2):
                    nc.scalar.activation(
                        prow_hw[:, h0 + bk * 4:h0 + bk * 4 + 4, :],
                        et[0:96, bk * 512:bk * 512 + 384],
                        EXP,
                    )

            # ---- stats: l = colsum+rowsum replicated; r = exp(-ln l) ----
            rrep = persist.tile([H, S], bf16, tag="rrep")
            for n in range(NSL):
                sl = slice(n * 512, (n + 1) * 512)
                lt = lps.tile([96, 512], f32, tag="lps")
                nc.tensor.matmul(
                    lt[:], ones_t[:], pcol[:, sl],
                    start=True, stop=False, skip_group_check=True,
                )
                nc.tensor.matmul(
                    lt[:], ones_t[:], prow[:, sl],
                    start=False, stop=True, skip_group_check=True,
                )
                tl = tsl.tile([96, 512], f32, tag="tln")
                nc.scalar.activation(tl[:], lt[:], LN)
                nc.scalar.activation(rrep[:, sl], tl[:], EXP, scale=-1.0)

            # ---- normalize P in place (gpsimd; keeps DVE free) ----
            for c0 in range(0, S, 2304):
                csl = slice(c0, c0 + 2304)
                nc.vector.tensor_tensor(
                    pcol[:, csl], pcol[:, csl], rrep[:, csl], MUL
                )
            for c0 in range(0, S, 2304):
                csl = slice(c0, c0 + 2304)
                nc.vector.tensor_tensor(
                    prow[:, csl], prow[:, csl], rrep[:, csl], MUL
                )

            # ---- Z accumulation ----
            z = [
                persist.tile([128, S], bf16, tag=f"z{kc}", name=f"z{kc}")
                for kc in range(2)
            ]
            z_wh = [zz[:].rearrange("p (h w) -> p w h", w=W) for zz in z]
            xtc = din[f"xtc{kvs}"][:].rearrange("p (w c) -> p w c", c=C)
            xtr = din[f"xtr{kvs}"][:].rearrange("p (h c) -> p h c", c=C)

            # column branch: per w, Z[c', h]; scatter w-strided into z
            for w0 in range(0, W, SLW):
                xs = slab.tile([H, SLW * 256], bf16, tag="xslab")
                nc.gpsimd.dma_start(xs[:], xtc[:, w0:w0 + SLW, :])
                for kc in range(2):
                    for j0 in range(0, SLW, 4):
                        zp = ps.tile([128, 512], f32, tag="ps")
                        for j in range(4):
                            wl = j0 + j
                            nc.tensor.matmul(
                                zp[:, j * 96:(j + 1) * 96],
                                xs[:, wl * 256 + kc * 128:wl * 256 + kc * 128 + 128],
                                pcol[:, (w0 + wl) * 96:(w0 + wl + 1) * 96],
                                start=True, stop=True, skip_group_check=True,
                            )
                        nc.scalar.activation(
                            z_wh[kc][:, w0 + j0:w0 + j0 + 4, :],
                            zp[:, 0:384], CPY,
                        )
            # row branch: per h, Z[c', w]; accumulate into z
            for h0 in range(0, H, SLW):
                xs = slab.tile([W, SLW * 256], bf16, tag="xslab")
                nc.gpsimd.dma_start(xs[:], xtr[:, h0:h0 + SLW, :])
                for kc in range(2):
                    for j0 in range(0, SLW, 4):
                        zp = ps.tile([128, 512], f32, tag="ps")
                        for j in range(4):
                            hl = j0 + j
                            nc.tensor.matmul(
                                zp[:, j * 96:(j + 1) * 96],
                                xs[:, hl * 256 + kc * 128:hl * 256 + kc * 128 + 128],
                                prow_hw[:, h0 + hl, :],
                                start=True, stop=True, skip_group_check=True,
                            )
                        zsl = z[kc][:, (h0 + j0) * 96:(h0 + j0 + 4) * 96]
                        nc.vector.tensor_tensor(zsl, zp[:, 0:384], zsl, ADD)

            # ---- final projection + residual + store ----
            for mc in range(2):
                for n0 in range(0, NSL, 6):
                    ob = obuf.tile([128, 3072], bf16, tag="ob")
                    rts = []
                    for hh in range(2):
                        rt = resl.tile([128, 1536], bf16, tag="res", name="res")
                        nc.sync.dma_start(
                            rt[:],
                            din[f"xa{kvs}"][
                                mc * 128:(mc + 1) * 128,
                                (n0 + hh * 3) * 512:(n0 + hh * 3 + 3) * 512,
                            ],
                        )
                        rts.append(rt)
                    for j in range(6):
                        n = n0 + j
                        sl = slice(n * 512, (n + 1) * 512)
                        jsl = slice(j * 512, (j + 1) * 512)
                        rsl = slice((j % 3) * 512, (j % 3 + 1) * 512)
                        op = ps.tile([128, 512], f32, tag="ps")
                        for kc in range(2):
                            nc.tensor.matmul(
                                op[:], wv_t[kvs][kc][mc], z[kc][:, sl],
                                start=(kc == 0), stop=(kc == 1),
                            )
                        nc.vector.tensor_tensor(
                            ob[:, jsl], op[:], rts[j // 3][:, rsl], ADD
                        )
                    nc.scalar.dma_start(
                        out[a, mc * 128:(mc + 1) * 128,
                            n0 * 512:(n0 + 6) * 512],
                        ob[:],
                    )


def prep_inputs(inputs):
    """Host-side per-core input prep (numpy)."""
    g = float(np.asarray(inputs["gamma"]).reshape(-1)[0])
    mask = np.zeros((H, H), np.float32)
    np.fill_diagonal(mask, NEG)
    wqk = np.concatenate(
        [inputs["wq0"].T, inputs["wk0"].T, inputs["wq1"].T, inputs["wk1"].T],
        axis=1,
    ).astype(BF)
    wv0 = (g * np.asarray(inputs["wv0"], np.float64)).T.astype(BF)
    wv1 = (g * np.asarray(inputs["wv1"], np.float64)).T.astype(BF)
    gb0 = g * np.asarray(inputs["bv0"], np.float64)
    gb1 = g * np.asarray(inputs["bv1"], np.float64)
    qkb = np.concatenate(
        [
            inputs["bq0"] - inputs["wq0"].astype(np.float64) @ gb0,
            inputs["bk0"] - inputs["wk0"].astype(np.float64) @ gb0,
            inputs["bq1"] - inputs["wq1"].astype(np.float64) @ gb1,
            inputs["bk1"] - inputs["wk1"].astype(np.float64) @ gb1,
        ]
    ).astype(np.float32)[:, None]
    maps = []
    for b in range(B):
        x0 = np.asarray(inputs["x0"][b], np.float32)
        x1 = np.asarray(inputs["x1"][b], np.float32)
        maps.append({
            "xa0": (x0 + np.float32(gb0[:, None, None])).reshape(C, S).astype(BF),
            "xa1": (x1 + np.float32(gb1[:, None, None])).reshape(C, S).astype(BF),
            "xtc0": np.ascontiguousarray(x0.transpose(1, 2, 0)).reshape(H, W * C).astype(BF),
            "xtr0": np.ascontiguousarray(x0.transpose(2, 1, 0)).reshape(W, H * C).astype(BF),
            "xtc1": np.ascontiguousarray(x1.transpose(1, 2, 0)).reshape(H, W * C).astype(BF),
            "xtr1": np.ascontiguousarray(x1.transpose(2, 1, 0)).reshape(W, H * C).astype(BF),
            "wqk": wqk, "wv0": wv0, "wv1": wv1, "qkb": qkb, "mask": mask,
        })
    return maps


def postprocess(results):
    cat0 = np.empty((B, C, H, W), np.float32)
    cat1 = np.empty((B, C, H, W), np.float32)
    for b in range(B):
        o = np.asarray(results[b]["out"]).astype(np.float32).reshape(2, C, H, W)
        cat0[b] = o[0]
        cat1[b] = o[1]
    return (cat0, cat1)


def kernel(**inputs):
    from concourse.bass_utils import run_bass_kernel_spmd

    if "nc" not in _CACHE:
        _CACHE["nc"] = build_nc()
    nc = _CACHE["nc"]
    maps = prep_inputs(inputs)
    res = run_bass_kernel_spmd(nc, maps, core_ids=list(range(B)))
    return postprocess(res.results)
